# revision 15
# baseline (speedup 1.0000x reference)
"""Trainium2 Bass kernel for a scalar-input LSTM (B=128, T=512, H=1024).

Strategy: model-parallel over the 4H gate dimension across 8 NeuronCores.
Core r owns hidden columns [128r, 128r+128) and the i/f/o/g gate rows for
those columns. Each step:
  PE:   gates[B=128, 512] = [x_t;1] @ [W_ih_shard; bias_shard]      (K=2 MM)
                          + sum_k hT[slot k] @ W_hh_shard^T[chunk k] (8 bf16 MMs)
  ACT:  sigmoid on [i|f|o] block (384 cols), tanh on g block
  DVE:  c = sig_f*c + sig_i*tanh_g ; h = sig_o*tanh(c) (bf16)
  PE:   transpose h -> h^T chunk; tiny N=1 MM accumulates y partial
  comm: per-step AllGather of each core's h^T chunk ([128,128] bf16) via
        collective_compute with DRAM bounce buffers (comm="cc", default).
        A lower-latency remote_dma_broadcast SBUF->SBUF path exists
        (comm="rdma") but the runtime ucode on this fleet crashes on the
        extended-ISA desc-gen, so it stays disabled.
The per-core y partials ([128, 512] f32) are summed on the host.
Per-core weight layouts are pre-permuted on the host so the SPMD program
needs no core-dependent addressing.

Self-contained: hardcodes shapes from the problem spec.
"""
import os
import sys

sys.path.insert(0, "/opt/trn_rl_repo")

import numpy as np
import ml_dtypes

B = 128
T = 512
H = 1024
NC = 8
HS = H // NC          # own hidden columns per core = 128
GS = 4 * HS           # own gate columns per core = 512
# gate column blocks in PSUM, in this order (so one sigmoid covers [i|f|o])
GATE_OFF = {"i": 0, "f": 128, "o": 256, "g": 384}
# PyTorch row offsets in W_hh/W_ih/bias: i, f, g, o
TORCH_OFF = {"i": 0, "f": H, "g": 2 * H, "o": 3 * H}

BF16 = ml_dtypes.bfloat16

# logical jax device index -> physical tpb index on the chip (for XOR routing).
# Discovered via probe on the target fleet; identity unless the driver maps
# logical NCs differently. Overridable for robustness.
PHYS_MAP = list(range(8))


def _gate_rows(r):
    """W_hh/W_ih/bias row indices for core r's 512 gate columns, PSUM order."""
    rows = np.empty(GS, dtype=np.int64)
    for gname, off in GATE_OFF.items():
        rows[off : off + 128] = TORCH_OFF[gname] + r * HS + np.arange(HS)
    return rows


def build_core_inputs(r, x, W_ih, W_hh, b_ih, b_hh, W_lin, comm="cc"):
    """Host-side data prep for core r (all numpy)."""
    rows = _gate_rows(r)
    bias = (b_ih + b_hh).astype(np.float32)
    # whh[k] = W_hh^T chunk for the sender whose tile lands in slot k.
    # rdma: sender s has phys(s) = phys(r) ^ k.  cc: slot k = sender k.
    pr = PHYS_MAP[r]
    inv = {PHYS_MAP[i]: i for i in range(NC)}
    whh = np.empty((NC, HS, GS), dtype=BF16)
    for k in range(NC):
        s = inv[pr ^ k] if comm == "rdma" else k
        whh[k] = W_hh[rows][:, s * HS : (s + 1) * HS].T.astype(BF16)
    wx2 = np.empty((2, GS), dtype=BF16)
    wx2[0] = W_ih[rows, 0].astype(BF16)
    wx2[1] = bias[rows].astype(BF16)
    # xsa[0, t, :] = x[:, t]; xsa[1, t, :] = 1  (K=2 stationary rows per step)
    xsa = np.ones((2, T, B), dtype=BF16)
    xsa[0] = x.T.astype(BF16)
    wlin = W_lin[0, r * HS : (r + 1) * HS].astype(BF16).reshape(HS, 1)
    ident = np.eye(128, dtype=BF16)
    return {"whh": whh, "wx2": wx2, "xsa": xsa, "wlin": wlin, "ident": ident}


def _patch_fast_compile():
    """Disable walrus's in-compile BIR simulation (verification-only pass) —
    it scales with unrolled step count and dominates compile time here."""
    from concourse import bass_utils

    if getattr(bass_utils, "_fastc", False):
        return
    orig = bass_utils.run_command

    def run_command_fast(argv, **kw):
        argv = [
            a.replace("--enable-birsim=true", "--enable-birsim=false")
            if isinstance(a, str)
            else a
            for a in argv
        ]
        return orig(argv, **kw)

    bass_utils.run_command = run_command_fast
    bass_utils._fastc = True


def build_bass(t_steps=T, detect_races=False, comm="cc"):
    import concourse.bass as bass
    import concourse.mybir as mybir
    from concourse import library_config

    fp32 = mybir.dt.float32
    bf16 = mybir.dt.bfloat16
    Sig = mybir.ActivationFunctionType.Sigmoid
    Tanh = mybir.ActivationFunctionType.Tanh
    mult = mybir.AluOpType.mult
    add = mybir.AluOpType.add

    nc = bass.Bass(
        target_bir_lowering=False, debug=False, detect_race_conditions=detect_races
    )

    whh_d = nc.declare_dram_parameter("whh", [NC, HS, GS], bf16, isOutput=False)
    wx2_d = nc.declare_dram_parameter("wx2", [2, GS], bf16, isOutput=False)
    xsa_d = nc.declare_dram_parameter("xsa", [2, T, B], bf16, isOutput=False)
    wlin_d = nc.declare_dram_parameter("wlin", [HS, 1], bf16, isOutput=False)
    id_d = nc.declare_dram_parameter("ident", [128, 128], bf16, isOutput=False)
    out_d = nc.declare_dram_parameter("out", [B, T], fp32, isOutput=True)
    if comm == "cc":
        bin_d = nc.dram_tensor("bounce_in", [128, B], bf16)
        bout_d = nc.dram_tensor("bounce_out", [NC, 128, B], bf16, addr_space="Shared")

    n_yblk = (t_steps + 127) // 128

    from contextlib import ExitStack

    with ExitStack() as ctx:
        whh_sb = ctx.enter_context(nc.sbuf_tensor("whh_sb", [128, NC, GS], bf16))
        wx2_sb = ctx.enter_context(nc.sbuf_tensor("wx2_sb", [2, GS], bf16))
        xsa_sb = ctx.enter_context(nc.sbuf_tensor("xsa_sb", [2, T, B], bf16))
        wlin_sb = ctx.enter_context(nc.sbuf_tensor("wlin_sb", [HS, 1], bf16))
        id_sb = ctx.enter_context(nc.sbuf_tensor("id_sb", [128, 128], bf16))
        hT_sb = ctx.enter_context(nc.sbuf_tensor("hT_sb", [128, 2, NC, B], bf16))
        sig_sb = ctx.enter_context(nc.sbuf_tensor("sig_sb", [B, 384], fp32))
        tg_sb = ctx.enter_context(nc.sbuf_tensor("tg_sb", [B, 128], fp32))
        t1_sb = ctx.enter_context(nc.sbuf_tensor("t1_sb", [B, 128], fp32))
        c_sb = ctx.enter_context(nc.sbuf_tensor("c_sb", [B, 128], fp32))
        tc_sb = ctx.enter_context(nc.sbuf_tensor("tc_sb", [B, 128], fp32))
        h_sb = ctx.enter_context(nc.sbuf_tensor("h_sb", [B, 128], bf16))
        hs_sb = ctx.enter_context(nc.sbuf_tensor("hs_sb", [128, B], bf16))
        y_sb = ctx.enter_context(nc.sbuf_tensor("y_sb", [B, T], fp32))
        g_ps = ctx.enter_context(nc.psum_tensor("g_ps", [B, 2, GS], fp32))
        tr_ps = ctx.enter_context(nc.psum_tensor("tr_ps", [128, 2, B], bf16))
        y_ps = ctx.enter_context(nc.psum_tensor("y_ps", [B, 128], fp32))
        din = ctx.enter_context(nc.semaphore("din"))
        rdy = ctx.enter_context(nc.semaphore("rdy"))
        rsem = ctx.enter_context(nc.semaphore("rsem"))
        lsem = ctx.enter_context(nc.semaphore("lsem"))
        msem = ctx.enter_context(nc.semaphore("msem"))
        a1 = ctx.enter_context(nc.semaphore("a1"))
        a2 = ctx.enter_context(nc.semaphore("a2"))
        a3 = ctx.enter_context(nc.semaphore("a3"))
        vc = ctx.enter_context(nc.semaphore("vc"))
        vh = ctx.enter_context(nc.semaphore("vh"))
        vs = ctx.enter_context(nc.semaphore("vs"))
        ptr = ctx.enter_context(nc.semaphore("ptr"))
        py = ctx.enter_context(nc.semaphore("py"))
        vyf = ctx.enter_context(nc.semaphore("vyf"))
        dsend = ctx.enter_context(nc.semaphore("dsend"))
        cc_s = ctx.enter_context(nc.semaphore("cc_s"))
        drecv = ctx.enter_context(nc.semaphore("drecv"))
        dout = ctx.enter_context(nc.semaphore("dout"))
        block = ctx.enter_context(nc.Block())
        N_IN_DMAS = NC + 4
        DIN_TOTAL = 16 * N_IN_DMAS  # calibrated vs sim; big xsa DMA may split

        @block.sync
        def _(sync):
            for k in range(NC):
                sync.dma_start(out=whh_sb[:, k, :], in_=whh_d[k, :, :]).then_inc(din, 16)
            sync.dma_start(out=wx2_sb[:, :], in_=wx2_d[:, :]).then_inc(din, 16)
            sync.dma_start(out=xsa_sb[:, :, :], in_=xsa_d[:, :, :]).then_inc(din, 16)
            sync.dma_start(out=wlin_sb[:, :], in_=wlin_d[:, :]).then_inc(din, 16)
            sync.dma_start(out=id_sb[:, :], in_=id_d[:, :]).then_inc(din, 16)
            sync.wait_ge(din, DIN_TOTAL)
            sync.sem_inc(rdy, 1)
            if comm == "cc":
                for t in range(t_steps - 1):
                    p2 = (t + 1) % 2
                    sync.wait_ge(vs, t + 1)
                    sync.dma_start(out=bin_d[:, :], in_=hs_sb[:, :]).then_inc(dsend, 16)
                    sync.wait_ge(cc_s, t + 1)
                    sync.dma_start(
                        out=hT_sb[:, p2, :, :],
                        in_=bout_d.ap().rearrange("c p w -> p c w"),
                    ).then_inc(drecv, 16)
            elif comm == "nocc":
                for t in range(t_steps - 1):
                    p2 = (t + 1) % 2
                    sync.wait_ge(vs, t + 1)
                    sync.dma_start(
                        out=hT_sb[:, p2, 0, :], in_=hs_sb[:, :]
                    ).then_inc(drecv, 16)

        @block.vector
        def _(vector):
            for t in range(t_steps):
                p = t % 2
                p2 = (t + 1) % 2
                blk = t // 128
                # wait gates + activations
                vector.wait_ge(a1, t + 1)
                vector.wait_ge(a2, t + 1)
                if t == 0:
                    vector.tensor_tensor(
                        c_sb[:, :], sig_sb[:, 0:128], tg_sb[:, :], mult
                    ).then_inc(vc, 1)
                else:
                    vector.tensor_tensor(t1_sb[:, :], sig_sb[:, 0:128], tg_sb[:, :], mult)
                    vector.tensor_tensor(c_sb[:, :], sig_sb[:, 128:256], c_sb[:, :], mult)
                    vector.drain()
                    vector.tensor_tensor(
                        c_sb[:, :], c_sb[:, :], t1_sb[:, :], add
                    ).then_inc(vc, 1)
                # h = sig_o * tanh(c)   (bf16 out)
                vector.wait_ge(a3, t + 1)
                if t >= 1:
                    vector.wait_ge(ptr, t)  # transpose(t-1) has read h_sb
                vector.tensor_tensor(
                    h_sb[:, :], sig_sb[:, 256:384], tc_sb[:, :], mult
                ).then_inc(vh, 1)
                # own h^T chunk -> send tile / gather slot 0
                vector.wait_ge(ptr, t + 1)
                if comm == "rdma":
                    if t >= 2:
                        vector.wait_ge(lsem, 112 * (t - 1))  # sends of t-2 done
                    vector.tensor_copy(hT_sb[:, p2, 0, :], tr_ps[:, p2, :]).then_inc(vs, 1)
                else:
                    if t >= 1:
                        vector.wait_ge(dsend, 16 * t)  # send DMA of t-1 done reading hs
                    vector.tensor_copy(hs_sb[:, :], tr_ps[:, p2, :]).then_inc(vs, 1)
                # y flush every 128 steps
                if t % 128 == 127 or t == t_steps - 1:
                    vector.wait_ge(py, t + 1)
                    w = t % 128 + 1
                    vector.tensor_copy(
                        y_sb[:, blk * 128 : blk * 128 + w], y_ps[:, 0:w]
                    ).then_inc(vyf, 1)

        @block.scalar
        def _(scalar):
            for t in range(t_steps):
                p = t % 2
                scalar.wait_ge(msem, t + 1)
                scalar.activation(sig_sb[:, :], g_ps[:, p, 0:384], Sig).then_inc(a1, 1)
                scalar.activation(tg_sb[:, :], g_ps[:, p, 384:512], Tanh).then_inc(a2, 1)
                scalar.wait_ge(vc, t + 1)
                scalar.activation(tc_sb[:, :], c_sb[:, :], Tanh).then_inc(a3, 1)

        @block.tensor
        def _(tensor):
            tensor.wait_ge(rdy, 1)
            for t in range(t_steps):
                p = t % 2
                p2 = (t + 1) % 2
                # K2 MM: x_t * W_ih + bias  (clears PSUM parity p)
                is_only = t == 0
                mm = tensor.matmul(
                    g_ps[:, p, :], xsa_sb[:, t, :], wx2_sb[:, :],
                    start=True, stop=is_only,
                )
                if t >= 1:
                    if comm == "rdma":
                        tensor.wait_ge(rsem, 14 * t)
                        tensor.wait_ge(vs, t)
                    else:
                        tensor.wait_ge(drecv, 16 * t)
                    for k in range(NC):
                        mm = tensor.matmul(
                            g_ps[:, p, :],
                            hT_sb[:, p, k, :],
                            whh_sb[:, k, :],
                            start=False,
                            stop=(k == NC - 1),
                        )
                mm.then_inc(msem, 1)
                # transpose h -> h^T (PSUM parity p2)
                tensor.wait_ge(vh, t + 1)
                tensor.transpose(tr_ps[:, p2, :], h_sb[:, :], id_sb[:, :]).then_inc(
                    ptr, 1
                )
                # y partial: y_ps[:, t%128] = hT_slot0 . wlin
                tensor.wait_ge(vs, t + 1)
                if t % 128 == 0 and t > 0:
                    tensor.wait_ge(vyf, t // 128)
                tensor.matmul(
                    y_ps[:, t % 128 : t % 128 + 1],
                    hT_sb[:, p2, 0, :] if comm == "rdma" else hs_sb[:, :],
                    wlin_sb[:, :],
                    start=True, stop=True,
                ).then_inc(py, 1)

        if comm == "nocc":
            pass
        elif comm == "rdma":

            @block.gpsimd
            def _(gp):
                gp.load_library(library_config.remote_dma)
                for t in range(t_steps - 1):  # last h never consumed remotely
                    p2 = (t + 1) % 2
                    if t >= 1:
                        gp.wait_ge(lsem, 112 * t)  # ring pacing
                    for k in range(1, NC):
                        rdests = [None] * NC
                        rdests[k] = (0, k)
                        gp.remote_dma_broadcast(
                            out_ap=hT_sb[:, p2, k, :],
                            in_ap=hT_sb[:, p2, 0, :],
                            remote_sem=rsem,
                            local_sem=lsem,
                            rdests=rdests,
                        )
                    gp.wait_ge(vs, t + 1)
                    gp.trigger_dma(count=7)
                if t_steps >= 2:
                    gp.wait_ge(lsem, 112 * (t_steps - 1))
                    gp.wait_ge(rsem, 14 * (t_steps - 1))
        else:

            @block.gpsimd
            def _(gp):
                for t in range(t_steps - 1):
                    gp.wait_ge(dsend, 16 * (t + 1))
                    gp.collective_compute(
                        "AllGather",
                        mybir.AluOpType.bypass,
                        ins=[bin_d.ap().opt()],
                        outs=[bout_d.ap().opt()],
                        replica_groups=[list(range(NC))],
                    ).then_inc(cc_s, 1)


        @block.sync
        def _(sync):
            sync.wait_ge(vyf, n_yblk)
            sync.dma_start(out=out_d[:, 0:t_steps], in_=y_sb[:, 0:t_steps]).then_inc(
                dout, 16
            )
            sync.wait_ge(dout, 16)

    return nc


_CACHED = {}


def _get_nc(t_steps=T):
    if t_steps not in _CACHED:
        _patch_fast_compile()
        nc = build_bass(t_steps)
        _CACHED[t_steps] = nc
    return _CACHED[t_steps]


def kernel(input, W_ih, W_hh, b_ih, b_hh, W_lin, b_lin):
    from concourse import bass_utils

    x = np.asarray(input, np.float32)
    W_ih = np.asarray(W_ih, np.float32)
    W_hh = np.asarray(W_hh, np.float32)
    b_ih = np.asarray(b_ih, np.float32)
    b_hh = np.asarray(b_hh, np.float32)
    W_lin = np.asarray(W_lin, np.float32)
    b_lin = np.asarray(b_lin, np.float32)

    in_maps = [
        build_core_inputs(r, x, W_ih, W_hh, b_ih, b_hh, W_lin) for r in range(NC)
    ]
    nc = _get_nc(T)
    res = bass_utils.run_bass_kernel_spmd(nc, in_maps, core_ids=list(range(NC)))
    y = np.zeros((B, T), np.float32)
    for r in range(NC):
        y += np.asarray(res.results[r]["out"], np.float32).reshape(B, T)
    y += b_lin[0]
    return y
